# revision 16
# baseline (speedup 1.0000x reference)
"""GAT-style ADSF message-passing kernel for 8 TRN2 NeuronCores.

Reference computation (N=2048, IN_F=512, OUT_F=256, H=4, NH=64):
    g  = (h @ W_lin).reshape(N, H, NH)
    el = g . a_src   (per-node, per-head scalar)   [N, H]
    er = g . a_dst                                  [N, H]
    e  = leaky_relu(el[i] + er[j], 0.2)             [N, N, H]
    masked softmax over j:  a = softmax(where(adj==0, -1e9, e), axis=1)
    s' = softmax(where(adj==0, -1e9, s), axis=1)
    new_attention = softmax(a + s', axis=1)    # masked entries contribute exp(0)=1
    out[i, hf]    = sum_j new_attention[i,j,h] * g[j,h,f]

Sharding: destination rows i split across 8 cores (256 rows each); g is
computed (replicated) on every core from the full h — no collectives.

Layout: i on partitions, j on the free dim. Masks are folded algebraically:
  exp(where(mask,-1e9,x)) == exp(x)*adj   (exp(-1e9) underflows to 0)
  t = exp(a + s') == exp(a)*exp(s'); masked entries: exp(0)*exp(0) = 1
Softmax denominators come free from tensor_tensor_reduce accumulators; the
j-contraction for aggregation uses PE transposes of t tiles + matmuls with a
fused ones-column (rhs = [g_h | 1]) so Z3 drops out of the same matmuls.
"""

import sys

if "/opt/trn_rl_repo" not in sys.path:
    sys.path.append("/opt/trn_rl_repo")

import numpy as np

N, IN_F, OUT_F, H = 2048, 512, 256, 4
NH = OUT_F // H  # 64
NEG_SLOPE = 0.2
M = 8            # cores
NI = N // M      # 256 destination rows per core
P = 128          # partitions
NB = NI // P     # 2 i-blocks per core
NJ = N // P      # 16 j-chunks
NC_ = IN_F // P  # 4 contraction chunks for g

last_exec_time_ns = None


def _build_nc():
    from contextlib import ExitStack

    import concourse.bacc as bacc
    import concourse.bass as bass
    import concourse.mybir as mybir
    import concourse.tile as tile
    from concourse import masks

    f32 = mybir.dt.float32
    Alu = mybir.AluOpType
    Act = mybir.ActivationFunctionType

    nc = bacc.Bacc("TRN2", target_bir_lowering=False, debug=False,
                   num_devices=M)
    h_d = nc.dram_tensor("h_full", [N, IN_F], f32, kind="ExternalInput")
    hb_d = nc.dram_tensor("h_blk", [NI, IN_F], f32, kind="ExternalInput")
    adj_d = nc.dram_tensor("adj", [NI, N], f32, kind="ExternalInput")
    s_d = nc.dram_tensor("s", [NI, N], f32, kind="ExternalInput")
    w_d = nc.dram_tensor("w", [IN_F, OUT_F], f32, kind="ExternalInput")
    aw_d = nc.dram_tensor("aw", [2 * NH], f32, kind="ExternalInput")
    out_d = nc.dram_tensor("out", [NI, OUT_F], f32, kind="ExternalOutput")
    er_d = nc.dram_tensor("er_scratch", [H, N], f32, kind="Internal")

    with ExitStack() as ctx:
        tc = ctx.enter_context(tile.TileContext(nc))
        singles = ctx.enter_context(tc.tile_pool(name="singles", bufs=1))
        pro = ctx.enter_context(tc.tile_pool(name="pro", bufs=1))
        prow = ctx.enter_context(tc.tile_pool(name="prow", bufs=2))
        store = ctx.enter_context(tc.tile_pool(name="store", bufs=1))
        mwork = ctx.enter_context(tc.tile_pool(name="mwork", bufs=2))
        epool = ctx.enter_context(tc.tile_pool(name="epool", bufs=3))
        zpool = ctx.enter_context(tc.tile_pool(name="zpool", bufs=2))
        ps_big = ctx.enter_context(tc.tile_pool(name="ps_big", bufs=2, space="PSUM"))
        ps_sm = ctx.enter_context(tc.tile_pool(name="ps_sm", bufs=1, space="PSUM"))
        ps_av = ctx.enter_context(tc.tile_pool(name="ps_av", bufs=2, space="PSUM"))

        # ---------------- prologue: identity, W, attn vectors ----------------
        ident = singles.tile([P, P], f32)
        masks.make_identity(nc, ident[:])

        w_sb = pro.tile([P, NC_, OUT_F], f32)  # W rows c*128+p
        nc.sync.dma_start(
            out=w_sb[:], in_=w_d[:, :].rearrange("(c p) o -> p c o", p=P)
        )

        # attn_w broadcast to all partitions: [128, 128] (cols 0:64 src, 64:128 dst)
        asd = pro.tile([P, 2 * NH], f32)
        aw_ap = aw_d[:]
        nc.sync.dma_start(
            out=asd[:],
            in_=bass.AP(tensor=aw_ap.tensor, offset=aw_ap.offset,
                        ap=[[0, P], *aw_ap.ap]),
        )

        # wsd[c][:, h] = sum_f W[c*128+p, h*64+f]*a_src[f]; cols 4..7 dst
        wsd = pro.tile([P, NC_, 2 * H], f32)
        junk = pro.tile([P, NH], f32)
        for c in range(NC_):
            for hh in range(H):
                nc.vector.tensor_mul(
                    junk[:], w_sb[:, c, hh * NH:(hh + 1) * NH], asd[:, 0:NH])
                nc.vector.tensor_reduce(
                    out=wsd[:, c, hh:hh + 1], in_=junk[:],
                    axis=mybir.AxisListType.X, op=Alu.add)
                nc.vector.tensor_mul(
                    junk[:], w_sb[:, c, hh * NH:(hh + 1) * NH],
                    asd[:, NH:2 * NH])
                nc.vector.tensor_reduce(
                    out=wsd[:, c, H + hh:H + hh + 1], in_=junk[:],
                    axis=mybir.AxisListType.X, op=Alu.add)

        # ---------------- g = h @ W  (+ er rows) over all 16 n-chunks --------
        # g_aug[:, n, h, 0:64] = g chunk, col 64 = 1.0 (fused ones column)
        g_aug = singles.tile([P, NJ, H, NH + 1], f32)
        nc.vector.memset(g_aug[:], 1.0)
        er_sb = pro.tile([H, N], f32)

        for n in range(NJ):
            h_n = prow.tile([P, IN_F], f32, tag="h_n")
            nc.sync.dma_start(out=h_n[:], in_=h_d[n * P:(n + 1) * P, :])
            hT_n = prow.tile([P, NC_, P], f32, tag="hT_n")
            for c in range(NC_):
                ps_t = ps_big.tile([P, P], f32, tag="ps_t")
                nc.tensor.transpose(ps_t[:], h_n[:, c * P:(c + 1) * P], ident[:])
                nc.any.tensor_copy(out=hT_n[:, c, :], in_=ps_t[:])
            ps_g = ps_sm.tile([P, OUT_F], f32, tag="ps_g")
            ps_e = ps_sm.tile([P, 2 * H], f32, tag="ps_e")
            for c in range(NC_):
                nc.tensor.matmul(ps_g[:], lhsT=hT_n[:, c, :], rhs=w_sb[:, c, :],
                                 start=(c == 0), stop=(c == NC_ - 1))
                nc.tensor.matmul(ps_e[:], lhsT=hT_n[:, c, :], rhs=wsd[:, c, :],
                                 start=(c == 0), stop=(c == NC_ - 1))
            nc.any.tensor_copy(
                out=g_aug[:, n, :, 0:NH],
                in_=ps_g[:].rearrange("p (h f) -> p h f", h=H),
            )
            elr_n = prow.tile([P, 2 * H], f32, tag="elr_n")
            nc.any.tensor_copy(out=elr_n[:], in_=ps_e[:])
            ps_eT = ps_sm.tile([H, P], f32, tag="ps_eT")
            nc.tensor.transpose(ps_eT[:], elr_n[:, H:2 * H], ident[:])
            nc.any.tensor_copy(out=er_sb[:, n * P:(n + 1) * P], in_=ps_eT[:])

        # er rows -> DRAM -> partition-broadcast back as ER[128, h, N]
        nc.sync.dma_start(out=er_d[:, :], in_=er_sb[:])
        er_bc = singles.tile([P, H, N], f32)
        for hh in range(H):
            row = er_d[hh, :]
            nc.sync.dma_start(
                out=er_bc[:, hh, :],
                in_=bass.AP(tensor=row.tensor, offset=row.offset,
                            ap=[[0, P], *row.ap]),
            )

        # own-rows el/er: from h_blk
        elr_own = singles.tile([P, NB, 2 * H], f32)
        for b in range(NB):
            hb_b = prow.tile([P, IN_F], f32, tag="h_n")
            nc.sync.dma_start(out=hb_b[:], in_=hb_d[b * P:(b + 1) * P, :])
            hbT = prow.tile([P, NC_, P], f32, tag="hT_n")
            for c in range(NC_):
                ps_t = ps_big.tile([P, P], f32, tag="ps_t")
                nc.tensor.transpose(ps_t[:], hb_b[:, c * P:(c + 1) * P], ident[:])
                nc.any.tensor_copy(out=hbT[:, c, :], in_=ps_t[:])
            ps_eo = ps_sm.tile([P, 2 * H], f32, tag="ps_e")
            for c in range(NC_):
                nc.tensor.matmul(ps_eo[:], lhsT=hbT[:, c, :], rhs=wsd[:, c, :],
                                 start=(c == 0), stop=(c == NC_ - 1))
            nc.any.tensor_copy(out=elr_own[:, b, :], in_=ps_eo[:])

        # ---------------- main: per i-block masked triple softmax ------------
        for b in range(NB):
            adj_b = mwork.tile([P, N], f32, tag="adj_b")
            s_b = mwork.tile([P, N], f32, tag="s_b")
            nc.sync.dma_start(out=adj_b[:], in_=adj_d[b * P:(b + 1) * P, :])
            nc.sync.dma_start(out=s_b[:], in_=s_d[b * P:(b + 1) * P, :])

            # madj = (adj - 1) * 1e9  (0 where edge, -1e9 where masked)
            madj = mwork.tile([P, N], f32, tag="madj")
            nc.vector.tensor_scalar(out=madj[:], in0=adj_b[:], scalar1=-1.0,
                                    scalar2=1e9, op0=Alu.add, op1=Alu.mult)

            z1 = zpool.tile([P, H], f32, tag="z1")
            z2 = zpool.tile([P, 1], f32, tag="z2")
            rz1 = zpool.tile([P, H], f32, tag="rz1")
            rz2 = zpool.tile([P, 1], f32, tag="rz2")
            rz3 = zpool.tile([P, H], f32, tag="rz3")

            em = {}
            for hh in range(H):
                e_h = epool.tile([P, N], f32, tag="e_h")
                nc.scalar.activation(e_h[:], er_bc[:, hh, :], Act.Lrelu,
                                     bias=elr_own[:, b, hh:hh + 1], scale=1.0,
                                     alpha=NEG_SLOPE)
                nc.vector.tensor_add(e_h[:], e_h[:], madj[:])
                em_h = store.tile([P, N], f32, tag=f"em{hh}")
                nc.scalar.activation(em_h[:], e_h[:], Act.Exp,
                                     accum_out=z1[:, hh:hh + 1])
                em[hh] = em_h

            sm_in = epool.tile([P, N], f32, tag="e_h")
            nc.vector.tensor_add(sm_in[:], s_b[:], madj[:])
            sm = store.tile([P, N], f32, tag="sm")
            nc.scalar.activation(sm[:], sm_in[:], Act.Exp,
                                 accum_out=z2[:, 0:1])

            nc.vector.reciprocal(rz1[:], z1[:])
            nc.vector.reciprocal(rz2[:], z2[:])

            # exp_sp = exp(s'_masked) ; masked entries -> exp(0) = 1
            sp = store.tile([P, N], f32, tag="sp")
            nc.scalar.activation(sp[:], sm[:], Act.Exp, scale=rz2[:, 0:1])

            out_sb = mwork.tile([P, OUT_F], f32, tag="out_sb")
            for hh in range(H):
                exp_a = epool.tile([P, N], f32, tag="e_h")
                nc.scalar.activation(exp_a[:], em[hh][:], Act.Exp,
                                     scale=rz1[:, hh:hh + 1])
                t_h = epool.tile([P, N], f32, tag="e_h")
                nc.vector.tensor_mul(t_h[:], exp_a[:], sp[:])
                ps_avh = ps_av.tile([P, NH + 1], f32, tag="ps_avh")
                for nn in range(NJ // 4):
                    ps_tt = ps_big.tile([P, 4 * P], f32, tag="ps_t")
                    for q in range(4):
                        nc.tensor.transpose(
                            ps_tt[:, q * P:(q + 1) * P],
                            t_h[:, (nn * 4 + q) * P:(nn * 4 + q + 1) * P],
                            ident[:],
                        )
                    tT = mwork.tile([P, 4 * P], f32, tag="tT")
                    nc.any.tensor_copy(out=tT[:], in_=ps_tt[:])
                    for q in range(4):
                        n_idx = nn * 4 + q
                        nc.tensor.matmul(
                            ps_avh[:], lhsT=tT[:, q * P:(q + 1) * P],
                            rhs=g_aug[:, n_idx, hh, :],
                            start=(n_idx == 0), stop=(n_idx == NJ - 1),
                        )
                # ones column of rhs accumulated sum_j t = Z3 in col NH
                nc.vector.reciprocal(rz3[:, hh:hh + 1], ps_avh[:, NH:NH + 1])
                nc.vector.tensor_scalar_mul(
                    out_sb[:, hh * NH:(hh + 1) * NH], ps_avh[:, 0:NH],
                    rz3[:, hh:hh + 1],
                )
            nc.sync.dma_start(out=out_d[b * P:(b + 1) * P, :], in_=out_sb[:])

    nc.compile()
    return nc


def kernel(h, adj_mat, s, W_lin, attn_w):
    global last_exec_time_ns
    from concourse.bass_utils import run_bass_kernel_spmd

    h = np.ascontiguousarray(np.asarray(h, dtype=np.float32))
    adj = np.ascontiguousarray(
        np.asarray(adj_mat, dtype=np.float32).reshape(N, N))
    s2 = np.ascontiguousarray(np.asarray(s, dtype=np.float32).reshape(N, N))
    W = np.ascontiguousarray(np.asarray(W_lin, dtype=np.float32))
    aw = np.ascontiguousarray(np.asarray(attn_w, dtype=np.float32))

    nc = _build_nc()
    in_maps = []
    for c in range(M):
        rows = slice(c * NI, (c + 1) * NI)
        in_maps.append({
            "h_full": h,
            "h_blk": np.ascontiguousarray(h[rows]),
            "adj": np.ascontiguousarray(adj[rows]),
            "s": np.ascontiguousarray(s2[rows]),
            "w": W,
            "aw": aw,
        })
    res = run_bass_kernel_spmd(nc, in_maps, core_ids=list(range(M)))
    last_exec_time_ns = res.exec_time_ns
    out = np.concatenate([res.results[c]["out"] for c in range(M)], axis=0)
    return out.astype(np.float32)
